# revision 1
# baseline (speedup 1.0000x reference)
"""GraphShiftOperator on 8 Trainium2 NeuronCores (raw Bass, explicit sync).

reference:
    out_deg = A.sum(1); in_deg = A.sum(0)
    forward = A.T * (1/(out_deg+eps))[None, :]   # = (diag(1/out_deg) @ A).T
    reverse = A  * (1/(in_deg+eps))[None, :]

Sharding: rows of A across 8 cores (1024 rows each).
  - out_deg local (row sums);  in_deg needs one 32KB AllReduce (col sums).
  - core s returns:
      fwd_scaled = A_s * d_out_inv[:, None]   (columns of `forward`;
                   host assembles forward = vstack(...).T, a free view)
      rev        = A_s * d_in_inv[None, :]    (rows of `reverse`)

Engine split per core:
  SP(sync)   all A-tile loads (16 x 4MB)
  ACT        all output stores (16 x 4MB)
  DVE        row sums, reciprocals, both elementwise multiplies
  PE         column sums (A_chunk.T @ ones accumulated in PSUM), transpose
  GPSIMD     identity build, collective + its bounce DMAs, d_in broadcast
"""

import sys

sys.path.insert(0, "/opt/trn_rl_repo")

from contextlib import ExitStack

import numpy as np

import concourse.bass as bass
from concourse import mybir
from concourse.bass_utils import run_bass_kernel_spmd

N = 8192
N_CORES = 8
ROWS = N // N_CORES          # 1024 rows per core
P = 128                      # partitions
NT = ROWS // P               # 8 row-tiles per core
NCC = N // P                 # 64 column chunks of 128
EPS = 1e-8
DT = mybir.dt.float32

_cache = {}


def _build():
    nc = bass.Bass(num_devices=N_CORES)

    a_in = nc.dram_tensor("a_shard", [ROWS, N], DT, kind="ExternalInput")
    fwd_out = nc.dram_tensor("fwd_scaled", [ROWS, N], DT, kind="ExternalOutput")
    rev_out = nc.dram_tensor("rev", [ROWS, N], DT, kind="ExternalOutput")
    cc_in = nc.dram_tensor("cc_in", [NCC, P], DT)
    cc_out = nc.dram_tensor("cc_out", [NCC, P], DT)

    ctx = ExitStack()
    with ctx:
        sem = lambda name: ctx.enter_context(nc.semaphore(name))
        li = sem("li")      # A-tile loads          (+16 each)
        ps0 = sem("ps0")    # cs_ps PSUM zeroed
        pe = sem("pe")      # PE col-sum tile done  (+1 per row-tile)
        on = sem("on")      # ones memset done
        idt = sem("idt")    # identity ready
        dv1 = sem("dv1")    # doi ready             (+1 per row-tile)
        am = sem("am")      # ACT fwd multiply done (+1 per row-tile)
        fo = sem("fo")      # fwd store done        (+16)
        cp1 = sem("cp1")    # colsum psum->sbuf copy
        tr = sem("tr")      # PE transpose done
        cp2 = sem("cp2")    # transpose psum->sbuf copy
        cci = sem("cci")    # cc_in bounce DMA done
        cc = sem("cc")      # collective done
        dinb = sem("dinb")  # d_in broadcast DMA done
        dv2 = sem("dv2")    # rev multiply done     (+1 per row-tile)
        ro = sem("ro")      # rev store done        (+16)

        sb = lambda name, shape: ctx.enter_context(nc.sbuf_tensor(name, shape, DT))
        a_sb = [sb(f"a{i}", [P, N]) for i in range(2)]       # input slots
        f_sb = [sb(f"f{i}", [P, N]) for i in range(2)]       # fwd slots
        din = sb("din", [P, N])
        ones = sb("ones", [P, 1])
        ident = sb("ident", [P, P])
        rs = sb("rs", [P, 1])
        doi = [sb(f"doi{i}", [P, 1]) for i in range(2)]
        cs_sb = sb("cs_sb", [P, NCC])
        csT = sb("csT", [NCC, P])

        cs_ps = ctx.enter_context(nc.psum_tensor("cs_ps", [P, NCC], DT))
        tr_ps = ctx.enter_context(nc.psum_tensor("tr_ps", [NCC, P], DT))

        with nc.Block() as block:

            @block.sync
            def _(sync):
                # phase-1 loads
                for t in range(NT):
                    if t >= 2:
                        sync.wait_ge(am, t - 1)    # ACT done with tile t-2
                        sync.wait_ge(pe, t - 1)    # PE done with tile t-2
                    sync.dma_start(
                        out=a_sb[t % 2][:], in_=a_in[t * P : (t + 1) * P, :]
                    ).then_inc(li, 16)
                # phase-2 loads
                for t in range(NT):
                    g = NT + t
                    if t < 2:
                        sync.wait_ge(am, g - 1)
                        sync.wait_ge(pe, g - 1)
                    else:
                        sync.wait_ge(ro, 16 * (t - 1))  # store of tile t-2 done
                    sync.dma_start(
                        out=a_sb[g % 2][:], in_=a_in[t * P : (t + 1) * P, :]
                    ).then_inc(li, 16)

            @block.scalar
            def _(scalar):
                for t in range(NT):
                    scalar.wait_ge(li, 16 * (t + 1))   # a_t loaded
                    scalar.wait_ge(dv1, t + 1)         # doi ready
                    if t >= 2:
                        scalar.wait_ge(fo, 16 * (t - 1))  # f slot free
                    scalar.mul(f_sb[t % 2][:], a_sb[t % 2][:], doi[t % 2][:])
                    scalar.drain().then_inc(am, 1)
                    scalar.dma_start(
                        out=fwd_out[t * P : (t + 1) * P, :], in_=f_sb[t % 2][:]
                    ).then_inc(fo, 16)
                for t in range(NT):
                    scalar.wait_ge(dv2, t + 1)
                    scalar.dma_start(
                        out=rev_out[t * P : (t + 1) * P, :],
                        in_=a_sb[(NT + t) % 2][:],
                    ).then_inc(ro, 16)

            @block.vector
            def _(vector):
                # NOTE: DVE results are not visible (even to DVE itself) until
                # an explicit drain; Tile inserts these automatically, raw
                # bass must do it by hand.
                vector.memset(ones[:], 1.0)
                vector.drain().then_inc(on, 1)
                # zero the col-sum accumulator: matmul start=True resets the
                # WHOLE PSUM bank (not just its own address), so interleaved
                # per-chunk groups can't use it; accumulate onto zeros instead.
                vector.memset(cs_ps[:], 0.0)
                vector.drain().then_inc(ps0, 1)
                for t in range(NT):
                    vector.wait_ge(li, 16 * (t + 1))
                    vector.reduce_sum(out=rs[:], in_=a_sb[t % 2][:], axis=mybir.AxisListType.X)
                    vector.drain()
                    vector.tensor_scalar_add(doi[t % 2][:], rs[:], EPS)
                    vector.drain()
                    if t >= 2:
                        vector.wait_ge(am, t - 1)  # doi slot free (ACT read t-2)
                    vector.reciprocal(doi[t % 2][:], doi[t % 2][:])
                    vector.drain().then_inc(dv1, 1)
                # local col sums -> natural order in csT
                vector.wait_ge(pe, NT)
                vector.tensor_copy(cs_sb[:], cs_ps[:])
                vector.drain().then_inc(cp1, 1)
                vector.wait_ge(tr, 1)
                vector.tensor_copy(csT[:], tr_ps[:])
                vector.drain().then_inc(cp2, 1)
                # d_in_inv = 1/(in_deg + eps), broadcast tile
                vector.wait_ge(dinb, 16)
                vector.tensor_scalar_add(din[:], din[:], EPS)
                vector.drain()
                vector.reciprocal(din[:], din[:])
                vector.drain()
                for t in range(NT):
                    vector.wait_ge(li, 16 * (NT + t + 1))
                    vector.tensor_mul(
                        a_sb[(NT + t) % 2][:], a_sb[(NT + t) % 2][:], din[:]
                    )
                    vector.drain().then_inc(dv2, 1)

            @block.tensor
            def _(tensor):
                tensor.wait_ge(on, 1)
                tensor.wait_ge(ps0, 1)
                for t in range(NT):
                    tensor.wait_ge(li, 16 * (t + 1))
                    for c in range(NCC):
                        mm = tensor.matmul(
                            cs_ps[:, c : c + 1],
                            a_sb[t % 2][:, c * P : (c + 1) * P],
                            ones[:],
                            start=False,
                            stop=(t == NT - 1),
                            skip_group_check=True,
                        )
                        if c == NCC - 1:
                            mm.then_inc(pe, 1)
                tensor.wait_ge(idt, 1)
                tensor.wait_ge(cp1, 1)
                tensor.transpose(tr_ps[:], cs_sb[:], ident[:]).then_inc(tr, 1)

            @block.gpsimd
            def _(gpsimd):
                gpsimd.memset(ident[:], 0.0)
                gpsimd.affine_select(
                    out=ident[:],
                    in_=ident[:],
                    compare_op=mybir.AluOpType.not_equal,
                    fill=1.0,
                    base=0,
                    pattern=[[-1, P]],
                    channel_multiplier=1,
                ).then_inc(idt, 1)
                gpsimd.wait_ge(cp2, 1)
                gpsimd.dma_start(out=cc_in[:], in_=csT[:]).then_inc(cci, 16)
                gpsimd.wait_ge(cci, 16)
                gpsimd.collective_compute(
                    "AllReduce",
                    mybir.AluOpType.add,
                    replica_groups=[list(range(N_CORES))],
                    ins=[cc_in[:]],
                    outs=[cc_out[:]],
                ).then_inc(cc, 1)
                gpsimd.wait_ge(cc, 1)
                gpsimd.dma_start(
                    out=din[:],
                    in_=bass.AP(cc_out, 0, [[0, P], [1, N]]),
                ).then_inc(dinb, 16)

    return nc


def kernel(adjacency_matrix: np.ndarray, _trace=False, _trace_kwargs=None):
    a = np.ascontiguousarray(adjacency_matrix, dtype=np.float32)
    assert a.shape == (N, N)

    if "nc" not in _cache:
        _cache["nc"] = _build()
    nc = _cache["nc"]

    in_maps = [
        {"a_shard": a[s * ROWS : (s + 1) * ROWS, :]} for s in range(N_CORES)
    ]
    kw = {}
    if _trace:
        kw = dict(trace=True, **(_trace_kwargs or {}))
    res = run_bass_kernel_spmd(nc, in_maps, list(range(N_CORES)), **kw)

    scaled = np.concatenate([r["fwd_scaled"] for r in res.results], axis=0)
    reverse = np.concatenate([r["rev"] for r in res.results], axis=0)
    forward = scaled.T
    if _trace:
        return (forward, reverse), res
    return forward, reverse



# revision 2
# speedup vs baseline: 2.5341x; 2.5341x over previous
"""GraphShiftOperator on 8 Trainium2 NeuronCores (raw Bass, explicit sync).

reference:
    out_deg = A.sum(1); in_deg = A.sum(0)
    forward = A.T * (1/(out_deg+eps))[None, :]   # = (diag(1/out_deg) @ A).T
    reverse = A  * (1/(in_deg+eps))[None, :]

v2 design (vs the fp32 two-pass v1 at 551 us):
  * All device I/O in bf16 (correctness gate is rel_err < 2e-2; bf16
    costs ~0.5%).  Halves every DMA transfer.
  * The whole 16 MB bf16 row-shard stays resident in SBUF, so A is
    read from HBM ONCE (v1 re-read it for the reverse pass).
    Per-core HBM traffic: 16 MB in + 32 MB out = 48 MB (vs 128 MB).
  * Column sums on PE with a ones[128,1] STATIONARY and 512-wide
    moving slices of A: 128 matmuls total (vs 512 chunk-stationary
    matmuls whose weight reloads made PE the 219 us critical path).
    Compact [1,512] PSUM rows come out in natural column order, so
    v1's PE transpose + its PSUM juggling are gone entirely.
    PSUM only holds 8 such banks -> columns 0..4095 accumulate while
    tiles stream in (phase A), columns 4096..8191 in a short PE-only
    re-pass over SBUF after the last load (phase B).
  * AllReduce (32 KB fp32) launches right after phase B and overlaps
    the tail of the forward stores.
  * d_in reciprocal is computed on the COMPACT [128,64] form before
    broadcast (v1 spent 51 us reciprocating the broadcast tile).

Per core:
  fwd_scaled = A_s * d_out_inv[:, None]   (host: forward = vstack.T)
  rev        = A_s * d_in_inv[None, :]    (rows of `reverse`)

Engine split:
  SP(sync)  A-tile loads, colsum bounce, d_in chain DMAs
  ACT       fwd multiplies, fwd + rev output stores
  DVE       row sums, reciprocals, PSUM->SBUF colsum copies, rev muls
  PE        column sums (ones-stationary matmuls)
  GPSIMD    AllReduce
"""

import sys

sys.path.insert(0, "/opt/trn_rl_repo")

from contextlib import ExitStack

import numpy as np
import ml_dtypes

import concourse.bass as bass
from concourse import mybir
from concourse.bass_utils import run_bass_kernel_spmd

N = 8192
N_CORES = 8
ROWS = N // N_CORES          # 1024 rows per core
P = 128                      # partitions
NT = ROWS // P               # 8 row-tiles per core
CH = 512                     # moving-dim chunk (PE max)
NCH = N // CH                # 16 column chunks
HALF = NCH // 2              # 8 chunks per PSUM phase
DT = mybir.dt.bfloat16
F32 = mybir.dt.float32

_cache = {}


def _build():
    nc = bass.Bass(num_devices=N_CORES)

    a_in = nc.dram_tensor("a_shard", [ROWS, N], DT, kind="ExternalInput")
    fwd_out = nc.dram_tensor("fwd_scaled", [ROWS, N], DT, kind="ExternalOutput")
    rev_out = nc.dram_tensor("rev", [ROWS, N], DT, kind="ExternalOutput")
    cc_in = nc.dram_tensor("cc_in", [N], F32)
    cc_out = nc.dram_tensor("cc_out", [N], F32)
    din_c = nc.dram_tensor("din_c", [N], DT)

    ctx = ExitStack()
    with ctx:
        sem = lambda name: ctx.enter_context(nc.semaphore(name))
        li = sem("li")      # A-tile loads            (+16 each)
        on = sem("on")      # ones memset done
        dv1 = sem("dv1")    # doi ready               (+1 per tile)
        am = sem("am")      # ACT fwd multiply done   (+1 per tile)
        fo = sem("fo")      # fwd store done          (+16 each)
        pA = sem("pA")      # phase-A colsum chunk    (+1 at stop)
        cpA = sem("cpA")    # phase-A psum copy done
        pB = sem("pB")      # phase-B colsum chunk    (+1 at stop)
        cpB = sem("cpB")    # phase-B psum copy done
        cci = sem("cci")    # colsum bounce DMA       (+16)
        cc = sem("cc")      # collective done
        cmpi = sem("cmpi")  # cc_out compact load     (+16)
        rcp = sem("rcp")    # compact reciprocal done
        dco = sem("dco")    # din_c store             (+16)
        dinb = sem("dinb")  # d_in broadcast          (+16)
        dv2 = sem("dv2")    # rev multiply done       (+1 per tile)
        ro = sem("ro")      # rev store done          (+16 each)

        sb = lambda name, shape, dt=DT: ctx.enter_context(
            nc.sbuf_tensor(name, shape, dt)
        )
        a_sb = [sb(f"a{t}", [P, N]) for t in range(NT)]   # resident shard
        f_sb = sb("f", [P, N])                            # fwd staging
        din = sb("din", [P, N])                           # d_in_inv bcast
        ones = sb("ones", [P, 1])
        rs = sb("rs", [P, 1], F32)
        doi = [sb(f"doi{t}", [P, 1], F32) for t in range(NT)]
        cs = sb("cs", [1, N], F32)                        # compact colsums
        cmp_f = sb("cmp_f", [P, N // P], F32)             # compact in_deg
        cmp_r = sb("cmp_r", [P, N // P], F32)
        cmp_b = sb("cmp_b", [P, N // P], DT)

        ps = ctx.enter_context(nc.psum_tensor("ps", [1, HALF * CH], F32))

        with nc.Block() as block:

            @block.sync
            def _(sync):
                for t in range(NT):
                    sync.dma_start(
                        out=a_sb[t][:], in_=a_in[t * P : (t + 1) * P, :]
                    ).then_inc(li, 16)
                # compact colsums -> collective input
                sync.wait_ge(cpA, 1)
                sync.wait_ge(cpB, 1)
                sync.dma_start(out=cc_in[:], in_=cs[:]).then_inc(cci, 16)
                # d_in chain: load compact, store bf16, broadcast
                sync.wait_ge(cc, 1)
                sync.dma_start(
                    out=cmp_f[:],
                    in_=bass.AP(cc_out, 0, [[N // P, P], [1, N // P]]),
                ).then_inc(cmpi, 16)
                sync.wait_ge(rcp, 1)
                sync.dma_start(
                    out=bass.AP(din_c, 0, [[N // P, P], [1, N // P]]),
                    in_=cmp_b[:],
                ).then_inc(dco, 16)
                sync.wait_ge(dco, 16)
                sync.dma_start(
                    out=din[:],
                    in_=bass.AP(din_c, 0, [[0, P], [1, N]]),
                ).then_inc(dinb, 16)

            @block.scalar
            def _(scalar):
                for t in range(NT):
                    scalar.wait_ge(li, 16 * (t + 1))
                    scalar.wait_ge(dv1, t + 1)
                    if t >= 1:
                        scalar.wait_ge(fo, 16 * t)   # f_sb free again
                    scalar.mul(f_sb[:], a_sb[t][:], doi[t][:])
                    scalar.drain().then_inc(am, 1)
                    scalar.dma_start(
                        out=fwd_out[t * P : (t + 1) * P, :], in_=f_sb[:]
                    ).then_inc(fo, 16)
                for t in range(NT):
                    scalar.wait_ge(dv2, t + 1)
                    scalar.dma_start(
                        out=rev_out[t * P : (t + 1) * P, :], in_=a_sb[t][:]
                    ).then_inc(ro, 16)

            @block.vector
            def _(vector):
                # NOTE: raw bass needs explicit drains for DVE results to
                # become visible (Tile inserts these automatically).
                vector.memset(ones[:], 1.0)
                vector.drain().then_inc(on, 1)
                for t in range(NT):
                    vector.wait_ge(li, 16 * (t + 1))
                    vector.reduce_sum(
                        out=rs[:], in_=a_sb[t][:], axis=mybir.AxisListType.X
                    )
                    vector.drain()
                    vector.reciprocal(doi[t][:], rs[:])
                    vector.drain().then_inc(dv1, 1)
                # compact colsum copies (free the PSUM banks for phase B)
                vector.wait_ge(pA, HALF)
                vector.tensor_copy(cs[0:1, 0 : HALF * CH], ps[:])
                vector.drain().then_inc(cpA, 1)
                vector.wait_ge(pB, HALF)
                vector.tensor_copy(cs[0:1, HALF * CH : N], ps[:])
                vector.drain().then_inc(cpB, 1)
                # d_in_inv on the compact [128,64] form
                vector.wait_ge(cmpi, 16)
                vector.reciprocal(cmp_r[:], cmp_f[:])
                vector.drain()
                vector.tensor_copy(cmp_b[:], cmp_r[:])
                vector.drain().then_inc(rcp, 1)
                # reverse multiplies, in place on the resident tiles
                vector.wait_ge(dinb, 16)
                for t in range(NT):
                    vector.wait_ge(am, t + 1)  # ACT done reading a_sb[t]
                    vector.tensor_mul(a_sb[t][:], a_sb[t][:], din[:])
                    vector.drain().then_inc(dv2, 1)

            @block.tensor
            def _(tensor):
                tensor.wait_ge(on, 1)
                # phase A: columns [0, HALF*CH) accumulate as tiles land
                for t in range(NT):
                    tensor.wait_ge(li, 16 * (t + 1))
                    for c in range(HALF):
                        mm = tensor.matmul(
                            ps[0:1, c * CH : (c + 1) * CH],
                            ones[:],
                            a_sb[t][:, c * CH : (c + 1) * CH],
                            start=(t == 0),
                            stop=(t == NT - 1),
                            skip_group_check=True,
                        )
                        if t == NT - 1:
                            mm.then_inc(pA, 1)
                # phase B: columns [HALF*CH, N) from resident SBUF
                tensor.wait_ge(cpA, 1)
                for t in range(NT):
                    for c in range(HALF):
                        mm = tensor.matmul(
                            ps[0:1, c * CH : (c + 1) * CH],
                            ones[:],
                            a_sb[t][:, HALF * CH + c * CH : HALF * CH + (c + 1) * CH],
                            start=(t == 0),
                            stop=(t == NT - 1),
                            skip_group_check=True,
                        )
                        if t == NT - 1:
                            mm.then_inc(pB, 1)

            @block.gpsimd
            def _(gpsimd):
                gpsimd.wait_ge(cci, 16)
                gpsimd.collective_compute(
                    "AllReduce",
                    mybir.AluOpType.add,
                    replica_groups=[list(range(N_CORES))],
                    ins=[cc_in[:]],
                    outs=[cc_out[:]],
                ).then_inc(cc, 1)

    return nc


def kernel(adjacency_matrix: np.ndarray, _trace=False, _trace_kwargs=None):
    a = np.asarray(adjacency_matrix)
    assert a.shape == (N, N)
    a_bf = np.ascontiguousarray(a).astype(ml_dtypes.bfloat16)

    if "nc" not in _cache:
        _cache["nc"] = _build()
    nc = _cache["nc"]

    in_maps = [
        {"a_shard": a_bf[s * ROWS : (s + 1) * ROWS, :]} for s in range(N_CORES)
    ]
    kw = {}
    if _trace:
        kw = dict(trace=True, **(_trace_kwargs or {}))
    res = run_bass_kernel_spmd(nc, in_maps, list(range(N_CORES)), **kw)

    scaled = np.concatenate([r["fwd_scaled"] for r in res.results], axis=0)
    reverse = np.concatenate([r["rev"] for r in res.results], axis=0)
    forward = scaled.T.astype(np.float32)
    reverse = reverse.astype(np.float32)
    if _trace:
        return (forward, reverse), res
    return forward, reverse


# revision 17
# speedup vs baseline: 3.1208x; 1.2315x over previous
"""GraphShiftOperator on 8 Trainium2 NeuronCores (raw Bass, explicit sync).

reference:
    out_deg = A.sum(1); in_deg = A.sum(0)
    forward = A.T * (1/(out_deg+eps))[None, :]   # = (diag(1/out_deg) @ A).T
    reverse = A  * (1/(in_deg+eps))[None, :]

v2 design (vs the fp32 two-pass v1 at 551 us):
  * All device I/O in bf16 (correctness gate is rel_err < 2e-2; bf16
    costs ~0.5%).  Halves every DMA transfer.
  * The whole 16 MB bf16 row-shard stays resident in SBUF, so A is
    read from HBM ONCE (v1 re-read it for the reverse pass).
    Per-core HBM traffic: 16 MB in + 32 MB out = 48 MB (vs 128 MB).
  * Column sums on PE with a ones[128,1] STATIONARY and 512-wide
    moving slices of A: 128 matmuls total (vs 512 chunk-stationary
    matmuls whose weight reloads made PE the 219 us critical path).
    Compact [1,512] PSUM rows come out in natural column order, so
    v1's PE transpose + its PSUM juggling are gone entirely.
    PSUM only holds 8 such banks -> columns 0..4095 accumulate while
    tiles stream in (phase A), columns 4096..8191 in a short PE-only
    re-pass over SBUF after the last load (phase B).
  * AllReduce (32 KB fp32) launches right after phase B and overlaps
    the tail of the forward stores.
  * d_in reciprocal is computed on the COMPACT [128,64] form before
    broadcast (v1 spent 51 us reciprocating the broadcast tile).

Per core:
  fwd_scaled = A_s * d_out_inv[:, None]   (host: forward = vstack.T)
  rev        = A_s * d_in_inv[None, :]    (rows of `reverse`)

Engine split:
  SP(sync)  A-tile loads, colsum bounce, d_in chain DMAs
  ACT       fwd multiplies, fwd + rev output stores
  DVE       row sums, reciprocals, PSUM->SBUF colsum copies, rev muls
  PE        column sums (ones-stationary matmuls)
  GPSIMD    AllReduce
"""

import sys

sys.path.insert(0, "/opt/trn_rl_repo")

from contextlib import ExitStack

import numpy as np
import ml_dtypes

import concourse.bass as bass
from concourse import mybir
from concourse.bass_utils import run_bass_kernel_spmd

N = 8192
N_CORES = 8
ROWS = N // N_CORES          # 1024 rows per core
P = 128                      # partitions
NT = ROWS // P               # 8 row-tiles per core
CH = 512                     # moving-dim chunk (PE max)
NCH = N // CH                # 16 column chunks
HALF = NCH // 2              # 8 chunks per PSUM phase
DT = mybir.dt.bfloat16
F32 = mybir.dt.float32

_cache = {}


def _build():
    nc = bass.Bass(num_devices=N_CORES)

    a_in = nc.dram_tensor("a_shard", [ROWS, N], DT, kind="ExternalInput")
    fwd_out = nc.dram_tensor("fwd_scaled", [ROWS, N], DT, kind="ExternalOutput")
    rev_out = nc.dram_tensor("rev", [ROWS, N], DT, kind="ExternalOutput")
    cc_in = nc.dram_tensor("cc_in", [N], F32)
    cc_out = nc.dram_tensor("cc_out", [N], F32)
    din_c = nc.dram_tensor("din_c", [N], DT)

    ctx = ExitStack()
    with ctx:
        sem = lambda name: ctx.enter_context(nc.semaphore(name))
        li = sem("li")      # A-tile loads            (+16 each)
        on = sem("on")      # ones memset + psum zero done
        dv1 = sem("dv1")    # doi ready               (+1 per tile)
        am = sem("am")      # ACT fwd multiply done   (+1 per tile)
        fo = sem("fo")      # fwd store done          (+16 each)
        pA = sem("pA")      # colsum chunk finished   (+1 at stop)
        cpA = sem("cpA")    # colsum psum copy done
        cci = sem("cci")    # colsum bounce DMA       (+16)
        cc = sem("cc")      # collective done
        cmpi = sem("cmpi")  # cc_out compact load     (+16)
        rcp = sem("rcp")    # compact reciprocal done
        dco = sem("dco")    # din_c store             (+16)
        dinb = sem("dinb")  # d_in broadcast          (+16)
        dv2 = sem("dv2")    # rev multiply done       (+1 per tile)
        ro = sem("ro")      # rev store done          (+16 each)

        sb = lambda name, shape, dt=DT: ctx.enter_context(
            nc.sbuf_tensor(name, shape, dt)
        )
        H = N // 2
        a_sb = [sb(f"a{t}", [P, N]) for t in range(NT)]   # resident shard
        f_sb = [sb(f"f{i}", [P, H]) for i in range(2)]    # fwd half-tiles
        junk = sb("junk", [P, H])                         # ttr main output
        din = sb("din", [P, N])                           # d_in_inv bcast
        ones = sb("ones", [P, 1])
        rs = sb("rs", [P, 1], F32)
        doi = [sb(f"doi{t}", [P, 1], F32) for t in range(NT)]
        cs = sb("cs", [65, HALF * CH], F32)               # compact colsums
        cmp_f = sb("cmp_f", [P, N // P], F32)             # compact in_deg
        cmp_b = sb("cmp_b", [P, N // P], DT)

        # chunk c lives at (partition (c//8)*64, bank c%8): all 16 colsum
        # chunks accumulate simultaneously in the 8 PSUM banks (PE output
        # base partition must be 0/32/64)
        ps = ctx.enter_context(nc.psum_tensor("ps", [P, HALF * CH], F32))

        with nc.Block() as block:

            @block.sync
            def _(sync):
                for t in range(NT):
                    sync.dma_start(
                        out=a_sb[t][:], in_=a_in[t * P : (t + 1) * P, :]
                    ).then_inc(li, 16)
                # compact colsums -> collective input (rows 0 and 64)
                sync.wait_ge(cpA, 1)
                sync.dma_start(
                    out=cc_in[0 : HALF * CH], in_=cs[0:1, :]
                ).then_inc(cci, 16)
                sync.dma_start(
                    out=cc_in[HALF * CH : N], in_=cs[64:65, :]
                ).then_inc(cci, 16)
                # d_in chain: load compact, store bf16, broadcast
                sync.wait_ge(cc, 1)
                sync.dma_start(
                    out=cmp_f[:],
                    in_=bass.AP(cc_out, 0, [[N // P, P], [1, N // P]]),
                ).then_inc(cmpi, 16)
                sync.wait_ge(rcp, 1)
                sync.dma_start(
                    out=bass.AP(din_c, 0, [[N // P, P], [1, N // P]]),
                    in_=cmp_b[:],
                ).then_inc(dco, 16)
                sync.wait_ge(dco, 16)
                sync.dma_start(
                    out=din[:],
                    in_=bass.AP(din_c, 0, [[0, P], [1, N]]),
                ).then_inc(dinb, 16)

            @block.scalar
            def _(scalar):
                # fwd multiply + store per HALF tile, double-buffered
                for t in range(NT):
                    scalar.wait_ge(li, 16 * (t + 1))
                    scalar.wait_ge(dv1, t + 1)
                    for hh in range(2):
                        h = 2 * t + hh
                        if h >= 2:
                            scalar.wait_ge(fo, 16 * (h - 1))  # buf h-2 stored
                        scalar.mul(
                            f_sb[h % 2][:],
                            a_sb[t][:, hh * H : (hh + 1) * H],
                            doi[t][:],
                        )
                        dr = scalar.drain()
                        if hh == 1:
                            dr.then_inc(am, 1)
                        scalar.dma_start(
                            out=fwd_out[t * P : (t + 1) * P, hh * H : (hh + 1) * H],
                            in_=f_sb[h % 2][:],
                        ).then_inc(fo, 16)
                for t in range(NT):
                    scalar.wait_ge(dv2, t + 1)
                    scalar.dma_start(
                        out=rev_out[t * P : (t + 1) * P, :], in_=a_sb[t][:]
                    ).then_inc(ro, 16)

            @block.vector
            def _(vector):
                # NOTE: raw bass needs explicit drains for DVE results to
                # become visible (Tile inserts these automatically).
                vector.memset(ones[:], 1.0)
                vector.memset(ps[:], 0.0)
                vector.drain().then_inc(on, 1)
                for t in range(NT):
                    vector.wait_ge(li, 16 * (t + 1))
                    # row sums: two bf16 TT folds (2 elem/cyc) + short
                    # reduce -- 5.1k cycles instead of 8.2k for a flat
                    # reduce.  Folds run in bf16 (error ~2 ulp per 4-sum,
                    # washed out by the fp32 final accumulate).
                    vector.tensor_add(junk[:], a_sb[t][:, 0:H], a_sb[t][:, H:N])
                    vector.drain()
                    vector.tensor_add(
                        junk[:, 0 : H // 2], junk[:, 0 : H // 2], junk[:, H // 2 : H]
                    )
                    vector.drain()
                    vector.reduce_sum(
                        out=rs[:],
                        in_=junk[:, 0 : H // 2],
                        axis=mybir.AxisListType.X,
                    )
                    vector.drain()
                    vector.reciprocal(doi[t][:], rs[:])
                    vector.drain().then_inc(dv1, 1)
                # compact colsum copy: partitions 0..64 in one parallel op
                # (only rows 0 and 64 carry data; lanes run in parallel so
                # copying the span costs the same 4k cycles)
                vector.wait_ge(pA, NCH)
                vector.tensor_copy(cs[:], ps[0:65, :])
                vector.drain().then_inc(cpA, 1)
                # d_in_inv on the compact [128,64] form, straight to bf16
                vector.wait_ge(cmpi, 16)
                with nc.allow_low_precision("d_in_inv rounds to bf16 anyway"):
                    vector.reciprocal(cmp_b[:], cmp_f[:])
                vector.drain().then_inc(rcp, 1)
                # reverse multiplies, in place on the resident tiles
                vector.wait_ge(dinb, 16)
                for t in range(NT):
                    vector.wait_ge(am, t + 1)  # ACT done reading a_sb[t]
                    vector.tensor_mul(a_sb[t][:], a_sb[t][:], din[:])
                    vector.drain().then_inc(dv2, 1)

            @block.tensor
            def _(tensor):
                tensor.wait_ge(on, 1)
                # all 16 column chunks accumulate as tiles land; chunk c
                # targets (partition c//8, bank c%8).  start=False always:
                # start=True zeroes the WHOLE bank, which would wipe the
                # co-resident chunk on the other partition row -- the
                # accumulator is memset once by DVE instead.
                for t in range(NT):
                    tensor.wait_ge(li, 16 * (t + 1))
                    for c in range(NCH):
                        row, b = (c // HALF) * 64, c % HALF
                        mm = tensor.matmul(
                            ps[row : row + 1, b * CH : (b + 1) * CH],
                            ones[:],
                            a_sb[t][:, c * CH : (c + 1) * CH],
                            start=False,
                            stop=(t == NT - 1),
                            skip_group_check=True,
                        )
                        if t == NT - 1:
                            mm.then_inc(pA, 1)

            @block.gpsimd
            def _(gpsimd):
                gpsimd.wait_ge(cci, 32)
                gpsimd.collective_compute(
                    "AllReduce",
                    mybir.AluOpType.add,
                    replica_groups=[list(range(N_CORES))],
                    ins=[cc_in[:]],
                    outs=[cc_out[:]],
                ).then_inc(cc, 1)

    return nc


def kernel(adjacency_matrix: np.ndarray, _trace=False, _trace_kwargs=None):
    a = np.asarray(adjacency_matrix)
    assert a.shape == (N, N)
    a_bf = np.ascontiguousarray(a).astype(ml_dtypes.bfloat16)

    if "nc" not in _cache:
        _cache["nc"] = _build()
    nc = _cache["nc"]

    in_maps = [
        {"a_shard": a_bf[s * ROWS : (s + 1) * ROWS, :]} for s in range(N_CORES)
    ]
    kw = {}
    if _trace:
        kw = dict(trace=True, **(_trace_kwargs or {}))
    res = run_bass_kernel_spmd(nc, in_maps, list(range(N_CORES)), **kw)

    scaled = np.concatenate([r["fwd_scaled"] for r in res.results], axis=0)
    reverse = np.concatenate([r["rev"] for r in res.results], axis=0)
    forward = scaled.T.astype(np.float32)
    reverse = reverse.astype(np.float32)
    if _trace:
        return (forward, reverse), res
    return forward, reverse
